# revision 11
# baseline (speedup 1.0000x reference)
"""Multi-head self-attention with RoPE on 8 Trainium2 NeuronCores.

Problem: B=2, S=2048, D_MODEL=2048, 16 heads x d_k=128, causal, RoPE on Q/K.

Sharding (hardcoded): core c -> batch b=c//4, head group g=c%4 (heads 4g..4g+3).
Data parallel on batch, tensor parallel on heads; q/k/v projections column-
sharded, output projection row-sharded with the partial sums reduced on host.

v2: all matmul operands bf16 (x, weights, Q/K/V, probabilities, OT) with f32
PSUM accumulation; rel-err budget is 2e-2 and bf16 rounding contributes
~0.2%.  bf16 runs the PE at 1 cycle/column at ANY output width, so the j=3
diagonal attention blocks narrow to 128 columns.  Three fused passes, each
keeping the PE queue dense:
  A: V + QK(pair0) projections (8 PSUM banks), x streamed bf16, startup DMAs
     interleaved (wv[et], x[et], wq[et], wk[et]) so the first matmul starts
     ~1us in instead of waiting 40us behind the whole weight prefetch.
  B: QK(pair1) projection (banks t0-t3) + pair-0 attention (t4-t7).
  C: pair-1 attention (t4-t7) + output projection (t0-t3), emitted with the
     out-proj lagging attention by one q-chunk so it never waits on drains.
Softmax tail per (head, q-chunk): den matmul -> reciprocal_approx_fast
(5x faster than reciprocal; den >= 1 so the approx is safe) -> gpsimd
partition_broadcast -> one DVE mul into OT.
"""

import sys

sys.path.insert(0, "/opt/trn_rl_repo")

import math
from contextlib import ExitStack

import ml_dtypes
import numpy as np

import concourse.bass as bass
import concourse.mybir as mybir
import concourse.tile as tile
from concourse import bacc
from concourse.bass_utils import run_bass_kernel_spmd

f32 = mybir.dt.float32
bf16 = mybir.dt.bfloat16

B = 2
S = 2048
D = 2048
H = 16
DK = 128
H_CORE = 4  # heads per core
DL = H_CORE * DK  # local feature dim 512
ET = D // 128  # 16 e-tiles (contraction over d_model)
QC = S // 512  # 4 q-chunks
THETA = 10000.0
SCALE = 1.0 / math.sqrt(DK)

N_CORES = 8


def _build():
    nc = bacc.Bacc("TRN2", target_bir_lowering=False, debug=False)

    xT_d = nc.dram_tensor("xT", [D, S], bf16, kind="ExternalInput")
    wqT_d = nc.dram_tensor("wqT", [D, DL], bf16, kind="ExternalInput")
    wkT_d = nc.dram_tensor("wkT", [D, DL], bf16, kind="ExternalInput")
    wvT_d = nc.dram_tensor("wvT", [D, DL], bf16, kind="ExternalInput")
    woT_d = nc.dram_tensor("woT", [DL, D], bf16, kind="ExternalInput")
    cosT_d = nc.dram_tensor("cosT", [64, S], bf16, kind="ExternalInput")
    sinT_d = nc.dram_tensor("sinT", [64, S], bf16, kind="ExternalInput")
    cmask_d = nc.dram_tensor("cmask", [4, 128, 512], bf16, kind="ExternalInput")
    outT_d = nc.dram_tensor("outT", [D, S], bf16, kind="ExternalOutput")

    Exp = mybir.ActivationFunctionType.Exp

    with tile.TileContext(nc) as tc:
      with tc.tile_pool(name="const", bufs=1) as const, \
           tc.tile_pool(name="persist", bufs=1) as persist, \
           tc.tile_pool(name="wp", bufs=1) as wp, \
           tc.tile_pool(name="xsp", bufs=18) as xsp, \
           tc.tile_pool(name="ropet", bufs=1) as ropet, \
           tc.tile_pool(name="psum", bufs=1, space="PSUM") as psum:

        V = [persist.tile([128, DL], bf16, tag=f"v{st}", name=f"v{st}")
             for st in range(ET)]
        QT = [persist.tile([DK, S], bf16, tag=f"qt{h}", name=f"qt{h}")
              for h in range(H_CORE)]
        KT = [persist.tile([DK, S], bf16, tag=f"kt{h}", name=f"kt{h}")
              for h in range(H_CORE)]
        OT = [persist.tile([DK, S], bf16, tag=f"ot{h}", name=f"ot{h}")
              for h in range(H_CORE)]

        # ---- startup: x tiles ride the gpsimd DMA queue, weights ride the
        # sync queue, so the first V matmul starts ~1.5us in and the two
        # queues stream concurrently.
        wv_sb = []
        wq_sb = {0: [], 1: []}
        wk_sb = {0: [], 1: []}
        xts0 = []
        for et in range(ET):
            xt = xsp.tile([128, 512], bf16, tag="xs", name="xs")
            nc.gpsimd.dma_start(xt[:], xT_d[et * 128 : (et + 1) * 128, 0:512])
            xts0.append(xt)
            wt = wp.tile([128, DL], bf16, tag=f"wv{et}", name="wv")
            nc.sync.dma_start(wt[:], wvT_d[et * 128 : (et + 1) * 128, :])
            wv_sb.append(wt)
            wqt = wp.tile([128, 256], bf16, tag=f"wq0_{et}", name="wq0")
            nc.sync.dma_start(wqt[:], wqT_d[et * 128 : (et + 1) * 128, 0:256])
            wq_sb[0].append(wqt)
            wkt = wp.tile([128, 256], bf16, tag=f"wk0_{et}", name="wk0")
            nc.sync.dma_start(wkt[:], wkT_d[et * 128 : (et + 1) * 128, 0:256])
            wk_sb[0].append(wkt)

        # constants (first needed at rope(qc0), ~27us of PE later)
        cos2 = const.tile([128, S], bf16, tag="cos2")
        sin2 = const.tile([128, S], bf16, tag="sin2")
        nc.sync.dma_start(cos2[0:64, :], cosT_d[:, :])
        nc.sync.dma_start(cos2[64:128, :], cosT_d[:, :])
        nc.sync.dma_start(sin2[0:64, :], sinT_d[:, :])
        nc.sync.dma_start(sin2[64:128, :], sinT_d[:, :])
        masks = []
        for j in range(4):
            w = 512 - 128 * j if j > 0 else 512
            mt = const.tile([128, w], bf16, tag=f"mask{j}", name=f"mask{j}")
            nc.sync.dma_start(mt[:], cmask_d[j, :, 512 - w : 512])
            masks.append(mt)
        ones = const.tile([128, 1], bf16, tag="ones")
        nc.vector.memset(ones[:], 1.0)

        def load_x(qc):
            xts = []
            for et in range(ET):
                xt = xsp.tile([128, 512], bf16, tag="xs", name="xs")
                nc.gpsimd.dma_start(
                    xt[:],
                    xT_d[et * 128 : (et + 1) * 128, qc * 512 : (qc + 1) * 512],
                )
                xts.append(xt)
            return xts

        def rope(dst, ev, od, qs):
            """ev/od: PSUM accumulators (128,512), rows [hA;hB]."""
            c = cos2[:, qs]
            sn = sin2[:, qs]
            m1 = ropet.tile([128, 512], f32, tag="m1")
            m2 = ropet.tile([128, 512], f32, tag="m2")
            n1 = ropet.tile([128, 512], f32, tag="n1")
            n2 = ropet.tile([128, 512], f32, tag="n2")
            nc.vector.tensor_mul(m1[:], ev[:], c)
            nc.vector.tensor_mul(m2[:], od[:], sn)
            nc.vector.tensor_mul(n1[:], ev[:], sn)
            nc.vector.tensor_mul(n2[:], od[:], c)
            nc.vector.tensor_sub(dst[0][0:64, qs], m1[0:64, :], m2[0:64, :])
            nc.vector.tensor_sub(dst[1][0:64, qs], m1[64:128, :], m2[64:128, :])
            nc.vector.tensor_add(dst[0][64:128, qs], n1[0:64, :], n2[0:64, :])
            nc.vector.tensor_add(
                dst[1][64:128, qs], n1[64:128, :], n2[64:128, :]
            )

        def qk_proj(p, qc, xts, tb):
            """QK projection matmuls for pair p, q-chunk qc into banks
            tb..tb+3; returns (qe, qo, ke, ko) PSUM tiles."""
            qs = slice(qc * 512, (qc + 1) * 512)
            qe = psum.tile([128, 512], f32, tag=f"t{tb}", name="qe")
            qo = psum.tile([128, 512], f32, tag=f"t{tb+1}", name="qo")
            ke = psum.tile([128, 512], f32, tag=f"t{tb+2}", name="ke")
            ko = psum.tile([128, 512], f32, tag=f"t{tb+3}", name="ko")
            for et in range(ET):
                xt = xts[et]
                nc.tensor.matmul(
                    qe[:], wq_sb[p][et][:, 0:128], xt[:],
                    start=(et == 0), stop=(et == ET - 1),
                )
                nc.tensor.matmul(
                    qo[:], wq_sb[p][et][:, 128:256], xt[:],
                    start=(et == 0), stop=(et == ET - 1),
                )
                nc.tensor.matmul(
                    ke[:], wk_sb[p][et][:, 0:128], xt[:],
                    start=(et == 0), stop=(et == ET - 1),
                )
                nc.tensor.matmul(
                    ko[:], wk_sb[p][et][:, 128:256], xt[:],
                    start=(et == 0), stop=(et == ET - 1),
                )
            return qe, qo, ke, ko

        with tc.tile_pool(name="ptp", bufs=8) as ptp, \
             tc.tile_pool(name="smallp", bufs=2) as smallp, \
             tc.tile_pool(name="stg", bufs=8) as stg:

            def attn(h, qc, td="t6", to="t7"):
                """Attention for head h over q-chunk qc (scores t4/t5,
                den bank td, oacc bank to).  pt in bf16; j=3 diagonal
                blocks are narrowed to 128 cols."""
                nkt = 4 * qc + 4
                LAG = 3
                den = psum.tile([1, 512], f32, tag=td, name="den")
                oacc = psum.tile([128, 512], f32, tag=to, name="oacc")
                qtt, ktt = QT[h], KT[h]
                pts = {}

                def consume(kt):
                    j = kt - 4 * qc
                    o = 128 * j if j > 0 else 0
                    cs = slice(o, 512)
                    pt = pts.pop(kt)
                    nc.tensor.matmul(
                        den[:, cs], ones[:], pt[:, cs],
                        start=(kt == 0), stop=(kt == nkt - 1),
                        skip_group_check=True,
                    )
                    nc.tensor.matmul(
                        oacc[:, cs],
                        V[kt][:, h * 128 : (h + 1) * 128],
                        pt[:, cs],
                        start=(kt == 0),
                        stop=(kt == nkt - 1),
                        skip_group_check=True,
                    )

                for kt in range(nkt):
                    j = kt - 4 * qc
                    o = 128 * j if j > 0 else 0
                    cs = slice(o, 512)
                    sps = psum.tile(
                        [128, 512], f32,
                        tag=("t4" if kt % 2 == 0 else "t5"), name="sps",
                    )
                    nc.tensor.matmul(
                        sps[:, cs],
                        ktt[:, kt * 128 : (kt + 1) * 128],
                        qtt[:, qc * 512 + o : (qc + 1) * 512],
                        start=True, stop=True,
                    )
                    pt = ptp.tile([128, 512], bf16, tag="pt")
                    nc.scalar.activation(pt[:, cs], sps[:, cs], Exp, scale=SCALE)
                    if j >= 0:
                        nc.vector.tensor_mul(pt[:, cs], pt[:, cs], masks[j][:])
                    pts[kt] = pt
                    if kt >= LAG:
                        consume(kt - LAG)
                for kt in range(max(0, nkt - LAG), nkt):
                    consume(kt)
                qs = slice(qc * 512, (qc + 1) * 512)
                rec = smallp.tile([1, 512], f32, tag="rec")
                nc.vector.reciprocal_approx_fast(out=rec[:], in_=den[:])
                bc = smallp.tile([128, 512], f32, tag="bc")
                nc.gpsimd.partition_broadcast(bc[:], rec[:])
                nc.vector.tensor_mul(OT[h][:, qs], oacc[:], bc[:])

            # ---- pass A: V + QK(pair0) --------------------------------
            for qc in range(QC):
                xts = xts0 if qc == 0 else load_x(qc)
                vacc = [
                    psum.tile([128, DL], f32, tag=f"t{i}", name=f"vacc{i}")
                    for i in range(4)
                ]
                for et in range(ET):
                    for sl in range(4):
                        nc.tensor.matmul(
                            vacc[sl][:],
                            xts[et][:, sl * 128 : (sl + 1) * 128],
                            wv_sb[et][:],
                            start=(et == 0),
                            stop=(et == ET - 1),
                        )
                qe, qo, ke, ko = qk_proj(0, qc, xts, 4)
                for sl in range(4):
                    nc.any.tensor_copy(V[qc * 4 + sl][:], vacc[sl][:])
                rope(QT[0:2], qe, qo, slice(qc * 512, (qc + 1) * 512))
                rope(KT[0:2], ke, ko, slice(qc * 512, (qc + 1) * 512))
                # pair-1 weights spread across pass A, behind x prefetch
                if qc in (1, 2):
                    wsb, w_d = (
                        (wq_sb[1], wqT_d) if qc == 1 else (wk_sb[1], wkT_d)
                    )
                    nm = "wq1" if qc == 1 else "wk1"
                    for et in range(ET):
                        wt = wp.tile([128, 256], bf16, tag=f"{nm}_{et}",
                                     name=nm)
                        nc.sync.dma_start(
                            wt[:], w_d[et * 128 : (et + 1) * 128, 256:512]
                        )
                        wsb.append(wt)

            # ---- pass B: QK(pair1) + attention(pair0) -----------------
            wo_t = {r: {} for r in range(4)}
            for qc in range(QC):
                xts = load_x(qc)
                qe, qo, ke, ko = qk_proj(1, qc, xts, 0)
                rope(QT[2:4], qe, qo, slice(qc * 512, (qc + 1) * 512))
                rope(KT[2:4], ke, ko, slice(qc * 512, (qc + 1) * 512))
                attn(0, qc)
                attn(1, qc)
                if qc == 0:
                    # wo subtiles for pass C: row-tile r covers woT rows
                    # r*128.. (head r), col-block cb covers e cols cb*512..
                    for cb in range(4):
                        for r in range(4):
                            wt = wp.tile([128, 512], bf16,
                                         tag=f"wo{r}_{cb}", name="wo")
                            nc.sync.dma_start(
                                wt[:],
                                woT_d[r * 128 : (r + 1) * 128,
                                      cb * 512 : (cb + 1) * 512],
                            )
                            wo_t[r][cb] = wt

            # ---- pass C: attention(pair1) + output projection ---------
            def outproj(qc):
                qs = slice(qc * 512, (qc + 1) * 512)
                for et in range(ET):
                    cb, co = et // 4, (et % 4) * 128
                    facc = psum.tile(
                        [128, 512], f32, tag=f"t{(et * QC + qc) % 2}",
                        name="facc",
                    )
                    for hh in range(H_CORE):
                        nc.tensor.matmul(
                            facc[:],
                            wo_t[hh][cb][:, co : co + 128],
                            OT[hh][:, qs],
                            start=(hh == 0),
                            stop=(hh == H_CORE - 1),
                        )
                    st = stg.tile([128, 512], bf16, tag="stg")
                    nc.vector.tensor_copy(st[:], facc[:])
                    nc.gpsimd.dma_start(
                        outT_d[et * 128 : (et + 1) * 128, qs], st[:]
                    )

            for qc in range(QC):
                attn(2, qc)
                attn(3, qc, td="t3", to="t2")
                if qc > 0:
                    outproj(qc - 1)
            outproj(QC - 1)

    return nc


_NC = None


def _get_nc():
    global _NC
    if _NC is None:
        _NC = _build()
        _NC.compile()
    return _NC


def _rope_perm_rows():
    """Row permutation applied to wq/wk for one core's 4 heads.

    Per head-pair p: [hA even dims, hB even dims, hA odd dims, hB odd dims]
    so the device sees even/odd deinterleaved, pair-stacked projections.
    Returns indices into the local (4*DK,) head-row block.
    """
    idx = []
    for p in range(2):
        ha, hb = 2 * p, 2 * p + 1
        idx.extend(ha * DK + np.arange(0, DK, 2))
        idx.extend(hb * DK + np.arange(0, DK, 2))
        idx.extend(ha * DK + np.arange(1, DK, 2))
        idx.extend(hb * DK + np.arange(1, DK, 2))
    return np.asarray(idx)


def _host_tables(positions):
    """cos/sin tables (64, S) float32, matching the fp32 reference math."""
    dim_idx = np.arange(0, DK, 2, dtype=np.float32)
    freqs = np.float32(THETA) ** (dim_idx / np.float32(DK))
    angles = positions.astype(np.float32)[:, None] / freqs[None, :]  # (S, 64)
    return (
        np.ascontiguousarray(np.cos(angles).T.astype(np.float32)),
        np.ascontiguousarray(np.sin(angles).T.astype(np.float32)),
    )


def _causal_masks():
    m = np.zeros((4, 128, 512), dtype=np.float32)
    p = np.arange(128)[:, None]
    f = np.arange(512)[None, :]
    for j in range(4):
        m[j] = (128 * j + p <= f).astype(np.float32)
    return m


def _make_in_maps(inputs):
    x = np.asarray(inputs["x"], dtype=np.float32)
    wq = np.asarray(inputs["wq"], dtype=np.float32)
    wk = np.asarray(inputs["wk"], dtype=np.float32)
    wv = np.asarray(inputs["wv"], dtype=np.float32)
    wo = np.asarray(inputs["wo"], dtype=np.float32)
    token_positions = np.asarray(inputs["token_positions"])

    perm = _rope_perm_rows()
    cmask = _causal_masks()
    bf = ml_dtypes.bfloat16

    in_maps = []
    for c in range(N_CORES):
        b = c // 4
        g = c % 4
        rows = slice(g * DL, (g + 1) * DL)
        cosT, sinT = _host_tables(token_positions[b])
        in_maps.append(
            {
                "xT": np.ascontiguousarray(x[b].T).astype(bf),
                "wqT": np.ascontiguousarray(wq[rows][perm].T).astype(bf),
                "wkT": np.ascontiguousarray(wk[rows][perm].T).astype(bf),
                "wvT": np.ascontiguousarray(wv[rows].T).astype(bf),
                "woT": np.ascontiguousarray(wo[:, rows].T).astype(bf),
                "cosT": cosT.astype(bf),
                "sinT": sinT.astype(bf),
                "cmask": cmask.astype(bf),
            }
        )
    return in_maps


def kernel(x, wq, wk, wv, wo, token_positions):
    nc = _get_nc()
    in_maps = _make_in_maps(
        {
            "x": x,
            "wq": wq,
            "wk": wk,
            "wv": wv,
            "wo": wo,
            "token_positions": token_positions,
        }
    )
    res = run_bass_kernel_spmd(nc, in_maps, list(range(N_CORES)))

    out = np.zeros((B, S, D), dtype=np.float32)
    for c in range(N_CORES):
        out[c // 4] += res.results[c]["outT"].T.astype(np.float32)
    return out


# revision 13
# speedup vs baseline: 1.0053x; 1.0053x over previous
"""Multi-head self-attention with RoPE on 8 Trainium2 NeuronCores.

Problem: B=2, S=2048, D_MODEL=2048, 16 heads x d_k=128, causal, RoPE on Q/K.

Sharding (hardcoded): core c -> batch b=c//4, head group g=c%4 (heads 4g..4g+3).
Data parallel on batch, tensor parallel on heads; q/k/v projections column-
sharded, output projection row-sharded with the partial sums reduced on host.

v2: all matmul operands bf16 (x, weights, Q/K/V, probabilities, OT) with f32
PSUM accumulation; rel-err budget is 2e-2 and bf16 rounding contributes
~0.2%.  bf16 runs the PE at 1 cycle/column at ANY output width, so the j=3
diagonal attention blocks narrow to 128 columns.  Three fused passes, each
keeping the PE queue dense:
  A: V + QK(pair0) projections (8 PSUM banks), x streamed bf16, startup DMAs
     interleaved (wv[et], x[et], wq[et], wk[et]) so the first matmul starts
     ~1us in instead of waiting 40us behind the whole weight prefetch.
  B: QK(pair1) projection (banks t0-t3) + pair-0 attention (t4-t7).
  C: pair-1 attention (t4-t7) + output projection (t0-t3), emitted with the
     out-proj lagging attention by one q-chunk so it never waits on drains.
Softmax tail per (head, q-chunk): den matmul -> reciprocal_approx_fast
(5x faster than reciprocal; den >= 1 so the approx is safe) -> gpsimd
partition_broadcast -> one DVE mul into OT.
"""

import sys

sys.path.insert(0, "/opt/trn_rl_repo")

import math
from contextlib import ExitStack

import ml_dtypes
import numpy as np

import concourse.bass as bass
import concourse.mybir as mybir
import concourse.tile as tile
from concourse import bacc
from concourse.bass_utils import run_bass_kernel_spmd

f32 = mybir.dt.float32
bf16 = mybir.dt.bfloat16

B = 2
S = 2048
D = 2048
H = 16
DK = 128
H_CORE = 4  # heads per core
DL = H_CORE * DK  # local feature dim 512
ET = D // 128  # 16 e-tiles (contraction over d_model)
QC = S // 512  # 4 q-chunks
THETA = 10000.0
SCALE = 1.0 / math.sqrt(DK)

N_CORES = 8


def _build():
    nc = bacc.Bacc("TRN2", target_bir_lowering=False, debug=False)

    xT_d = nc.dram_tensor("xT", [D, S], bf16, kind="ExternalInput")
    wqT_d = nc.dram_tensor("wqT", [D, DL], bf16, kind="ExternalInput")
    wkT_d = nc.dram_tensor("wkT", [D, DL], bf16, kind="ExternalInput")
    wvT_d = nc.dram_tensor("wvT", [D, DL], bf16, kind="ExternalInput")
    woT_d = nc.dram_tensor("woT", [DL, D], bf16, kind="ExternalInput")
    cosT_d = nc.dram_tensor("cosT", [64, S], bf16, kind="ExternalInput")
    sinT_d = nc.dram_tensor("sinT", [64, S], bf16, kind="ExternalInput")
    cmask_d = nc.dram_tensor("cmask", [4, 128, 512], bf16, kind="ExternalInput")
    outT_d = nc.dram_tensor("outT", [D, S], bf16, kind="ExternalOutput")

    Exp = mybir.ActivationFunctionType.Exp

    with tile.TileContext(nc) as tc:
      with tc.tile_pool(name="const", bufs=1) as const, \
           tc.tile_pool(name="persist", bufs=1) as persist, \
           tc.tile_pool(name="wp", bufs=1) as wp, \
           tc.tile_pool(name="xsp", bufs=18) as xsp, \
           tc.tile_pool(name="ropet", bufs=1) as ropet, \
           tc.tile_pool(name="psum", bufs=1, space="PSUM") as psum:

        V = [persist.tile([128, DL], bf16, tag=f"v{st}", name=f"v{st}")
             for st in range(ET)]
        QT = [persist.tile([DK, S], bf16, tag=f"qt{h}", name=f"qt{h}")
              for h in range(H_CORE)]
        KT = [persist.tile([DK, S], bf16, tag=f"kt{h}", name=f"kt{h}")
              for h in range(H_CORE)]
        OT = [persist.tile([DK, S], bf16, tag=f"ot{h}", name=f"ot{h}")
              for h in range(H_CORE)]

        # ---- startup: interleave x/wv/wq0/wk0 per e-tile on the sync queue
        # so the first V matmul starts ~1.5us in.  gpsimd stays a pure
        # partition-broadcast engine: mixing DMAs onto it forces a ~8us
        # UNLOAD_LIB/LOAD_LIB swap between its DMA and broadcast libraries.
        wv_sb = []
        wq_sb = {0: [], 1: []}
        wk_sb = {0: [], 1: []}
        xts0 = []
        for et in range(ET):
            xt = xsp.tile([128, 512], bf16, tag="xs", name="xs")
            nc.sync.dma_start(xt[:], xT_d[et * 128 : (et + 1) * 128, 0:512])
            xts0.append(xt)
            wt = wp.tile([128, DL], bf16, tag=f"wv{et}", name="wv")
            nc.sync.dma_start(wt[:], wvT_d[et * 128 : (et + 1) * 128, :])
            wv_sb.append(wt)
            wqt = wp.tile([128, 256], bf16, tag=f"wq0_{et}", name="wq0")
            nc.sync.dma_start(wqt[:], wqT_d[et * 128 : (et + 1) * 128, 0:256])
            wq_sb[0].append(wqt)
            wkt = wp.tile([128, 256], bf16, tag=f"wk0_{et}", name="wk0")
            nc.sync.dma_start(wkt[:], wkT_d[et * 128 : (et + 1) * 128, 0:256])
            wk_sb[0].append(wkt)

        # constants (first needed at rope(qc0), ~27us of PE later)
        cos2 = const.tile([128, S], bf16, tag="cos2")
        sin2 = const.tile([128, S], bf16, tag="sin2")
        nc.sync.dma_start(cos2[0:64, :], cosT_d[:, :])
        nc.sync.dma_start(cos2[64:128, :], cosT_d[:, :])
        nc.sync.dma_start(sin2[0:64, :], sinT_d[:, :])
        nc.sync.dma_start(sin2[64:128, :], sinT_d[:, :])
        masks = []
        for j in range(4):
            w = 512 - 128 * j if j > 0 else 512
            mt = const.tile([128, w], bf16, tag=f"mask{j}", name=f"mask{j}")
            nc.sync.dma_start(mt[:], cmask_d[j, :, 512 - w : 512])
            masks.append(mt)
        ones = const.tile([128, 1], bf16, tag="ones")
        nc.vector.memset(ones[:], 1.0)

        def load_x(qc):
            xts = []
            for et in range(ET):
                xt = xsp.tile([128, 512], bf16, tag="xs", name="xs")
                nc.sync.dma_start(
                    xt[:],
                    xT_d[et * 128 : (et + 1) * 128, qc * 512 : (qc + 1) * 512],
                )
                xts.append(xt)
            return xts

        def rope(dst, ev, od, qs):
            """ev/od: PSUM accumulators (128,512), rows [hA;hB]."""
            c = cos2[:, qs]
            sn = sin2[:, qs]
            m1 = ropet.tile([128, 512], f32, tag="m1")
            m2 = ropet.tile([128, 512], f32, tag="m2")
            n1 = ropet.tile([128, 512], f32, tag="n1")
            n2 = ropet.tile([128, 512], f32, tag="n2")
            nc.vector.tensor_mul(m1[:], ev[:], c)
            nc.vector.tensor_mul(m2[:], od[:], sn)
            nc.vector.tensor_mul(n1[:], ev[:], sn)
            nc.vector.tensor_mul(n2[:], od[:], c)
            nc.vector.tensor_sub(dst[0][0:64, qs], m1[0:64, :], m2[0:64, :])
            nc.vector.tensor_sub(dst[1][0:64, qs], m1[64:128, :], m2[64:128, :])
            nc.vector.tensor_add(dst[0][64:128, qs], n1[0:64, :], n2[0:64, :])
            nc.vector.tensor_add(
                dst[1][64:128, qs], n1[64:128, :], n2[64:128, :]
            )

        def qk_proj(p, qc, xts, tb):
            """QK projection matmuls for pair p, q-chunk qc into banks
            tb..tb+3; returns (qe, qo, ke, ko) PSUM tiles."""
            qs = slice(qc * 512, (qc + 1) * 512)
            qe = psum.tile([128, 512], f32, tag=f"t{tb}", name="qe")
            qo = psum.tile([128, 512], f32, tag=f"t{tb+1}", name="qo")
            ke = psum.tile([128, 512], f32, tag=f"t{tb+2}", name="ke")
            ko = psum.tile([128, 512], f32, tag=f"t{tb+3}", name="ko")
            for et in range(ET):
                xt = xts[et]
                nc.tensor.matmul(
                    qe[:], wq_sb[p][et][:, 0:128], xt[:],
                    start=(et == 0), stop=(et == ET - 1),
                )
                nc.tensor.matmul(
                    qo[:], wq_sb[p][et][:, 128:256], xt[:],
                    start=(et == 0), stop=(et == ET - 1),
                )
                nc.tensor.matmul(
                    ke[:], wk_sb[p][et][:, 0:128], xt[:],
                    start=(et == 0), stop=(et == ET - 1),
                )
                nc.tensor.matmul(
                    ko[:], wk_sb[p][et][:, 128:256], xt[:],
                    start=(et == 0), stop=(et == ET - 1),
                )
            return qe, qo, ke, ko

        with tc.tile_pool(name="ptp", bufs=8) as ptp, \
             tc.tile_pool(name="smallp", bufs=2) as smallp, \
             tc.tile_pool(name="stg", bufs=8) as stg:

            def attn(h, qc, td="t6", to="t7"):
                """Attention for head h over q-chunk qc (scores t4/t5,
                den bank td, oacc bank to).  pt in bf16; j=3 diagonal
                blocks are narrowed to 128 cols."""
                nkt = 4 * qc + 4
                LAG = 3
                den = psum.tile([1, 512], f32, tag=td, name="den")
                oacc = psum.tile([128, 512], f32, tag=to, name="oacc")
                qtt, ktt = QT[h], KT[h]
                pts = {}

                def consume(kt):
                    j = kt - 4 * qc
                    o = 128 * j if j > 0 else 0
                    cs = slice(o, 512)
                    pt = pts.pop(kt)
                    nc.tensor.matmul(
                        den[:, cs], ones[:], pt[:, cs],
                        start=(kt == 0), stop=(kt == nkt - 1),
                        skip_group_check=True,
                    )
                    nc.tensor.matmul(
                        oacc[:, cs],
                        V[kt][:, h * 128 : (h + 1) * 128],
                        pt[:, cs],
                        start=(kt == 0),
                        stop=(kt == nkt - 1),
                        skip_group_check=True,
                    )

                for kt in range(nkt):
                    j = kt - 4 * qc
                    o = 128 * j if j > 0 else 0
                    cs = slice(o, 512)
                    sps = psum.tile(
                        [128, 512], f32,
                        tag=("t4" if kt % 2 == 0 else "t5"), name="sps",
                    )
                    nc.tensor.matmul(
                        sps[:, cs],
                        ktt[:, kt * 128 : (kt + 1) * 128],
                        qtt[:, qc * 512 + o : (qc + 1) * 512],
                        start=True, stop=True,
                    )
                    pt = ptp.tile([128, 512], bf16, tag="pt")
                    nc.scalar.activation(pt[:, cs], sps[:, cs], Exp, scale=SCALE)
                    if j >= 0:
                        nc.vector.tensor_mul(pt[:, cs], pt[:, cs], masks[j][:])
                    pts[kt] = pt
                    if kt >= LAG:
                        consume(kt - LAG)
                for kt in range(max(0, nkt - LAG), nkt):
                    consume(kt)
                qs = slice(qc * 512, (qc + 1) * 512)
                rec = smallp.tile([1, 512], f32, tag="rec")
                nc.vector.reciprocal_approx_fast(out=rec[:], in_=den[:])
                bc = smallp.tile([128, 512], f32, tag="bc")
                nc.gpsimd.partition_broadcast(bc[:], rec[:])
                nc.vector.tensor_mul(OT[h][:, qs], oacc[:], bc[:])

            # ---- pass A: V + QK(pair0) --------------------------------
            for qc in range(QC):
                xts = xts0 if qc == 0 else load_x(qc)
                vacc = [
                    psum.tile([128, DL], f32, tag=f"t{i}", name=f"vacc{i}")
                    for i in range(4)
                ]
                for et in range(ET):
                    for sl in range(4):
                        nc.tensor.matmul(
                            vacc[sl][:],
                            xts[et][:, sl * 128 : (sl + 1) * 128],
                            wv_sb[et][:],
                            start=(et == 0),
                            stop=(et == ET - 1),
                        )
                qe, qo, ke, ko = qk_proj(0, qc, xts, 4)
                for sl in range(4):
                    nc.any.tensor_copy(V[qc * 4 + sl][:], vacc[sl][:])
                rope(QT[0:2], qe, qo, slice(qc * 512, (qc + 1) * 512))
                rope(KT[0:2], ke, ko, slice(qc * 512, (qc + 1) * 512))
                # pair-1 weights spread across pass A, behind x prefetch
                if qc in (1, 2):
                    wsb, w_d = (
                        (wq_sb[1], wqT_d) if qc == 1 else (wk_sb[1], wkT_d)
                    )
                    nm = "wq1" if qc == 1 else "wk1"
                    for et in range(ET):
                        wt = wp.tile([128, 256], bf16, tag=f"{nm}_{et}",
                                     name=nm)
                        nc.sync.dma_start(
                            wt[:], w_d[et * 128 : (et + 1) * 128, 256:512]
                        )
                        wsb.append(wt)

            # ---- pass B: QK(pair1) + attention(pair0) -----------------
            wo_t = {r: {} for r in range(4)}
            for qc in range(QC):
                xts = load_x(qc)
                qe, qo, ke, ko = qk_proj(1, qc, xts, 0)
                rope(QT[2:4], qe, qo, slice(qc * 512, (qc + 1) * 512))
                rope(KT[2:4], ke, ko, slice(qc * 512, (qc + 1) * 512))
                attn(0, qc)
                attn(1, qc)
                if qc == 0:
                    # wo subtiles for pass C: row-tile r covers woT rows
                    # r*128.. (head r), col-block cb covers e cols cb*512..
                    for cb in range(4):
                        for r in range(4):
                            wt = wp.tile([128, 512], bf16,
                                         tag=f"wo{r}_{cb}", name="wo")
                            nc.sync.dma_start(
                                wt[:],
                                woT_d[r * 128 : (r + 1) * 128,
                                      cb * 512 : (cb + 1) * 512],
                            )
                            wo_t[r][cb] = wt

            # ---- pass C: attention(pair1) + output projection ---------
            def outproj(qc):
                qs = slice(qc * 512, (qc + 1) * 512)
                for et in range(ET):
                    cb, co = et // 4, (et % 4) * 128
                    facc = psum.tile(
                        [128, 512], f32, tag=f"t{(et * QC + qc) % 2}",
                        name="facc",
                    )
                    for hh in range(H_CORE):
                        nc.tensor.matmul(
                            facc[:],
                            wo_t[hh][cb][:, co : co + 128],
                            OT[hh][:, qs],
                            start=(hh == 0),
                            stop=(hh == H_CORE - 1),
                        )
                    st = stg.tile([128, 512], bf16, tag="stg")
                    nc.vector.tensor_copy(st[:], facc[:])
                    nc.sync.dma_start(
                        outT_d[et * 128 : (et + 1) * 128, qs], st[:]
                    )

            for qc in range(QC):
                attn(2, qc)
                attn(3, qc, td="t3", to="t2")
                if qc > 0:
                    outproj(qc - 1)
            outproj(QC - 1)

    return nc


_NC = None


def _get_nc():
    global _NC
    if _NC is None:
        _NC = _build()
        _NC.compile()
    return _NC


def _rope_perm_rows():
    """Row permutation applied to wq/wk for one core's 4 heads.

    Per head-pair p: [hA even dims, hB even dims, hA odd dims, hB odd dims]
    so the device sees even/odd deinterleaved, pair-stacked projections.
    Returns indices into the local (4*DK,) head-row block.
    """
    idx = []
    for p in range(2):
        ha, hb = 2 * p, 2 * p + 1
        idx.extend(ha * DK + np.arange(0, DK, 2))
        idx.extend(hb * DK + np.arange(0, DK, 2))
        idx.extend(ha * DK + np.arange(1, DK, 2))
        idx.extend(hb * DK + np.arange(1, DK, 2))
    return np.asarray(idx)


def _host_tables(positions):
    """cos/sin tables (64, S) float32, matching the fp32 reference math."""
    dim_idx = np.arange(0, DK, 2, dtype=np.float32)
    freqs = np.float32(THETA) ** (dim_idx / np.float32(DK))
    angles = positions.astype(np.float32)[:, None] / freqs[None, :]  # (S, 64)
    return (
        np.ascontiguousarray(np.cos(angles).T.astype(np.float32)),
        np.ascontiguousarray(np.sin(angles).T.astype(np.float32)),
    )


def _causal_masks():
    m = np.zeros((4, 128, 512), dtype=np.float32)
    p = np.arange(128)[:, None]
    f = np.arange(512)[None, :]
    for j in range(4):
        m[j] = (128 * j + p <= f).astype(np.float32)
    return m


def _make_in_maps(inputs):
    x = np.asarray(inputs["x"], dtype=np.float32)
    wq = np.asarray(inputs["wq"], dtype=np.float32)
    wk = np.asarray(inputs["wk"], dtype=np.float32)
    wv = np.asarray(inputs["wv"], dtype=np.float32)
    wo = np.asarray(inputs["wo"], dtype=np.float32)
    token_positions = np.asarray(inputs["token_positions"])

    perm = _rope_perm_rows()
    cmask = _causal_masks()
    bf = ml_dtypes.bfloat16

    in_maps = []
    for c in range(N_CORES):
        b = c // 4
        g = c % 4
        rows = slice(g * DL, (g + 1) * DL)
        cosT, sinT = _host_tables(token_positions[b])
        in_maps.append(
            {
                "xT": np.ascontiguousarray(x[b].T).astype(bf),
                "wqT": np.ascontiguousarray(wq[rows][perm].T).astype(bf),
                "wkT": np.ascontiguousarray(wk[rows][perm].T).astype(bf),
                "wvT": np.ascontiguousarray(wv[rows].T).astype(bf),
                "woT": np.ascontiguousarray(wo[:, rows].T).astype(bf),
                "cosT": cosT.astype(bf),
                "sinT": sinT.astype(bf),
                "cmask": cmask.astype(bf),
            }
        )
    return in_maps


def kernel(x, wq, wk, wv, wo, token_positions):
    nc = _get_nc()
    in_maps = _make_in_maps(
        {
            "x": x,
            "wq": wq,
            "wk": wk,
            "wv": wv,
            "wo": wo,
            "token_positions": token_positions,
        }
    )
    res = run_bass_kernel_spmd(nc, in_maps, list(range(N_CORES)))

    out = np.zeros((B, S, D), dtype=np.float32)
    for c in range(N_CORES):
        out[c // 4] += res.results[c]["outT"].T.astype(np.float32)
    return out
